# revision 1
# baseline (speedup 1.0000x reference)
"""Causal self-attention Bass kernel for TRN2, 8 NeuronCores.

Sharding: data-parallel over batch (B=4) x tensor-parallel over head halves
(2 groups of 8 heads) = 8 shards, Megatron-style. Each core computes its
batch's qkv projection for its 8 heads, causal attention, and a partial
output projection (its heads' rows of W_proj). The host sums the two
partials per batch and adds b_proj.

All matmul operands are fp16 (full-rate 1 cycle/row on the PE, fp32 PSUM
accumulation; fp16's 10 mantissa bits keep end-to-end rel err ~4e-4).

Layouts per core:
  xt   = x[b].T (fp16)                 (C=1024, T=2048)
  wqk  = [Wq_half | Wk_half] (fp16)    (1024, 1024)
  wv   = Wv_half (fp16)                (1024, 512)
  wp   = W_proj[512*h2:+512, :] (fp16) (512, 1024)
  QT/KT tiles [128, 512] fp16: partitions = d + 64*(h%2) for head pair h//2
  V tiles [128, 8, 65] fp16: per s-chunk, 8 heads x (64 V cols + ones col)
  scores^T [s,t] (2-head row-packed, K=64, diag cols clipped) -> ACT exp
  -> DVE causal mask-mul -> PV matmul M=65 -> O^T[d,t] + Z row in PSUM
  -> recip_approx_fast + gpsimd partition_broadcast -> normalized OCT (SBUF)
  proj: out[t, c] = sum_hd OCT[hd, t] * wp[hd, c]

Emission interleaves phase-1 QK chunks and output-projection chunks into
the ACT-heavy attention rounds so the PE static order has filler work.
"""

import math
import os

import numpy as np

import concourse.bass as bass
import concourse.mybir as mybir
from concourse import bacc
from concourse.tile import TileContext

F32 = mybir.dt.float32
F32R = mybir.dt.float32r
BF16 = mybir.dt.bfloat16
F16 = mybir.dt.float16

N_EMBD = 1024
N_HEAD = 16
D = 64
B = 4
T = 2048
N_CORES = 8
PAIRS = 4          # head pairs per core (8 heads)
TJ = T // 512      # 512-wide t super-chunks
SJ = T // 128      # 128-wide s chunks
SCALE = 1.0 / math.sqrt(D)

_CACHE = {}


def _build():
    nc = bacc.Bacc()

    xt_d = nc.declare_dram_parameter("xt", [N_EMBD, T], F16, isOutput=False)
    wqk_d = nc.declare_dram_parameter("wqk", [N_EMBD, 1024], F16, isOutput=False)
    wv_d = nc.declare_dram_parameter("wv", [N_EMBD, 512], F16, isOutput=False)
    wp_d = nc.declare_dram_parameter("wp", [512, N_EMBD], F16, isOutput=False)
    bqk_d = nc.declare_dram_parameter("bqk", [128, 8], F32, isOutput=False)
    bv_d = nc.declare_dram_parameter("bv", [1, 512], F16, isOutput=False)
    out_d = nc.declare_dram_parameter("out_p", [T, N_EMBD], F32, isOutput=True)


    with TileContext(nc) as tc:
        with (
            tc.tile_pool(name="const", bufs=1) as cpool,
            tc.tile_pool(name="w", bufs=1) as wpool,
            tc.tile_pool(name="xt", bufs=20) as xpool,
            tc.tile_pool(name="qkt", bufs=1) as qkpool,
            tc.tile_pool(name="v", bufs=1) as vpool,
            tc.tile_pool(name="e", bufs=8) as epool,
            tc.tile_pool(name="octp", bufs=1) as octpool,
            tc.tile_pool(name="misc", bufs=3) as mpool,
            tc.tile_pool(name="outp", bufs=3) as opool,
            tc.tile_pool(name="ps", bufs=2, space="PSUM") as pspool,
            tc.tile_pool(name="pv", bufs=4, space="PSUM") as pvpool,
        ):
            # ---- constants ----
            ones_f = cpool.tile([1, 128], F32, tag="ones_f")
            nc.vector.memset(ones_f, 1.0)
            ones_r = cpool.tile([1, 128], F16, tag="ones_r")
            nc.vector.tensor_copy(ones_r, ones_f)
            ones8 = cpool.tile([128, 8], F32, tag="ones8")
            nc.vector.memset(ones8, 1.0)
            bqk_t = cpool.tile([128, 8], F32, tag="bqk")
            nc.sync.dma_start(out=bqk_t, in_=bqk_d[:, :])
            bv_t = cpool.tile([1, 512], F16, tag="bv")
            nc.sync.dma_start(out=bv_t, in_=bv_d[:, :])

            # bvb = b_v broadcast to [128, 512] via K=1 matmul
            ps_bvb = pvpool.tile([128, 512], F32, tag="pv")
            nc.tensor.matmul(
                ps_bvb, lhsT=ones_r[0:1, :], rhs=bv_t, start=True, stop=True
            )
            bvb = cpool.tile([128, 512], F32, tag="bvb")
            nc.vector.tensor_copy(bvb, ps_bvb)

            # causal masks for the 4 diagonal offsets: keep where f - p - 128k >= 0
            masks = []
            for k in range(4):
                mk = cpool.tile([128, 512], F16, tag=f"mask{k}")
                nc.vector.memset(mk, 1.0)
                nc.gpsimd.affine_select(
                    out=mk, in_=mk, compare_op=mybir.AluOpType.is_ge, fill=0.0,
                    base=-128 * k, pattern=[[1, 512]], channel_multiplier=-1,
                )
                masks.append(mk)

            # ---- weights (wv first: the V pass is the first consumer) ----
            wqk = []
            wv = []
            wp = []
            def load_xt(tj):
                xts = []
                for c in range(8):
                    t = xpool.tile([128, 512], F16, tag="xt")
                    nc.sync.dma_start(
                        out=t,
                        in_=xt_d[128 * c : 128 * c + 128, 512 * tj : 512 * tj + 512],
                    )
                    xts.append(t)
                return xts

            xts0 = []
            for c in range(8):
                t = wpool.tile([128, 512], F16, tag=f"wv{c}")
                nc.sync.dma_start(out=t, in_=wv_d[128 * c : 128 * c + 128, :])
                wv.append(t)
                t2_ = xpool.tile([128, 512], F16, tag="xt")
                nc.sync.dma_start(
                    out=t2_, in_=xt_d[128 * c : 128 * c + 128, 0:512]
                )
                xts0.append(t2_)

            for c in range(8):
                t = wpool.tile([128, 1024], F16, tag=f"wqk{c}")
                nc.sync.dma_start(out=t, in_=wqk_d[128 * c : 128 * c + 128, :])
                wqk.append(t)
            for p in range(PAIRS):
                t = wpool.tile([128, 1024], F16, tag=f"wp{p}")
                nc.sync.dma_start(out=t, in_=wp_d[128 * p : 128 * p + 128, :])
                wp.append(t)

            QT = [[None] * TJ for _ in range(PAIRS)]
            KT = [[None] * TJ for _ in range(PAIRS)]
            V = [None] * SJ
            OCT = [[None] * TJ for _ in range(PAIRS)]

            def v_chunk(tj, sj, xts):
                s_idx = 4 * tj + sj
                pv = pvpool.tile([128, 512], F32, tag="pv")
                for c in range(8):
                    nc.tensor.matmul(
                        pv,
                        lhsT=xts[c][:, 128 * sj : 128 * sj + 128],
                        rhs=wv[c],
                        start=(c == 0),
                        stop=(c == 7),
                    )
                vt = vpool.tile([128, 8, 65], F16, tag=f"v{s_idx}")
                nc.vector.tensor_add(
                    vt[:, :, 0:64],
                    pv.rearrange("p (h d) -> p h d", h=8),
                    bvb.rearrange("p (h d) -> p h d", h=8),
                )
                nc.vector.tensor_copy(
                    vt[:, :, 64:65], ones8.rearrange("p (h o) -> p h o", h=8)
                )
                V[s_idx] = vt

            def qk_chunk(tj, n, xts):
                ps = pspool.tile([128, 1024], F32, tag="ps")
                for c in range(8):
                    nc.tensor.matmul(
                        ps[:, 0:512],
                        lhsT=wqk[c][:, 128 * n : 128 * n + 128],
                        rhs=xts[c],
                        start=(c == 0),
                        stop=(c == 7),
                    )
                dst = qkpool.tile([128, 512], F16, tag=f"qk{n}_{tj}")
                nc.vector.tensor_scalar_add(dst, ps[:, 0:512], bqk_t[:, n : n + 1])
                if n < 4:
                    QT[n][tj] = dst
                else:
                    KT[n - 4][tj] = dst

            def attention_block(pair, tcj):
                nk = 4 * tcj + 4  # kept s-chunks (causal)
                pv1 = pvpool.tile([128, 512], F32, tag="pv")
                pv2 = pvpool.tile([128, 512], F32, tag="pv")
                qt = QT[pair][tcj]
                for si in range(nk):
                    kt = KT[pair][si // 4]
                    koff = 128 * (si % 4)
                    f0 = max(0, 128 * (si - 4 * tcj))  # cols < f0 are fully masked
                    ps = pspool.tile([128, 1024], F32, tag="ps")
                    psv = ps.rearrange("p (g f) -> p g f", g=2)
                    nc.tensor.matmul(
                        psv[:, 0, f0:512],
                        lhsT=kt[0:64, koff : koff + 128],
                        rhs=qt[0:64, f0:512],
                        start=True,
                        stop=True,
                        tile_position=(0, 0),
                    )
                    nc.tensor.matmul(
                        psv[:, 1, f0:512],
                        lhsT=kt[64:128, koff : koff + 128],
                        rhs=qt[64:128, f0:512],
                        start=True,
                        stop=True,
                        tile_position=(64, 0),
                    )
                    et = epool.tile([128, 2, 512], F16, tag="e")
                    nc.scalar.activation(
                        out=et[:, :, f0:512],
                        in_=psv[:, :, f0:512],
                        func=mybir.ActivationFunctionType.Exp,
                        scale=SCALE,
                    )
                    if si >= 4 * tcj:
                        k = si - 4 * tcj
                        nc.vector.tensor_mul(
                            et[:, 0, f0:512], et[:, 0, f0:512], masks[k][:, f0:512]
                        )
                        nc.vector.tensor_mul(
                            et[:, 1, f0:512], et[:, 1, f0:512], masks[k][:, f0:512]
                        )
                    h1 = 2 * pair
                    h2 = 2 * pair + 1
                    nc.tensor.matmul(
                        pv1[0:65, f0:512],
                        lhsT=V[si][:, h1, :],
                        rhs=et[:, 0, f0:512],
                        start=(si == 0),
                        stop=(si == nk - 1),
                    )
                    nc.tensor.matmul(
                        pv2[0:65, f0:512],
                        lhsT=V[si][:, h2, :],
                        rhs=et[:, 1, f0:512],
                        start=(si == 0),
                        stop=(si == nk - 1),
                    )
                # normalize: O[d, t] / Z[t]  (Z in psum row 0)
                oct_t = octpool.tile([128, 512], F16, tag=f"oct{pair}_{tcj}")
                OCT[pair][tcj] = oct_t
                for g, pv in enumerate((pv1, pv2)):
                    rz = mpool.tile([1, 512], F32, tag="rz")
                    nc.vector.tensor_copy(rz, pv[64:65, :])
                    nc.vector.reciprocal_approx_fast(out=rz, in_=rz)
                    # broadcast [1,512] -> [64,512] (gpsimd partition broadcast)
                    rzb = mpool.tile([64, 512], F32, tag="rzb")
                    nc.gpsimd.partition_broadcast(rzb, rz)
                    nc.vector.tensor_mul(
                        oct_t[64 * g : 64 * g + 64, :], pv[0:64, :], rzb
                    )

            def proj_chunk(t2, evict_on_act=False):
                tcj, k = t2 // 4, t2 % 4
                octc = [OCT[pair][tcj][:, 128 * k : 128 * k + 128] for pair in range(PAIRS)]
                for cj in range(2):
                    po = pvpool.tile([128, 512], F32, tag="pv")
                    for pair in range(PAIRS):
                        nc.tensor.matmul(
                            po,
                            lhsT=octc[pair],
                            rhs=wp[pair][:, 512 * cj : 512 * cj + 512],
                            start=(pair == 0),
                            stop=(pair == 3),
                        )
                    ot = opool.tile([128, 512], F32, tag="out")
                    if evict_on_act:
                        # tail chunks: ScalarE is idle once the last exp is done
                        nc.scalar.copy(ot, po)
                    else:
                        nc.vector.tensor_copy(ot, po)
                    nc.sync.dma_start(
                        out=out_d[
                            128 * t2 : 128 * t2 + 128, 512 * cj : 512 * cj + 512
                        ],
                        in_=ot,
                    )

            # ---- pipelined emission: phase1, attention, interleaved proj ----
            # proj for t-range of round tj-1 is striped across round tj's
            # attention blocks so the PE has filler work while ACT runs exp.
            xts_cur = xts0
            for tj in range(TJ):
                for sj in range(4):
                    v_chunk(tj, sj, xts_cur)
                if tj == 0:
                    for p in range(PAIRS):
                        qk_chunk(0, p, xts_cur)
                        qk_chunk(0, 4 + p, xts_cur)
                xts_next = load_xt(tj + 1) if tj + 1 < TJ else None
                for pair in range(PAIRS):
                    attention_block(pair, tj)
                    if tj == 2:
                        proj_chunk(0 + pair)
                    elif tj == 3:
                        proj_chunk(4 + pair)
                        proj_chunk(8 + pair)
                    if tj + 1 < TJ:
                        qk_chunk(tj + 1, pair, xts_next)
                        qk_chunk(tj + 1, 4 + pair, xts_next)
                xts_cur = xts_next
            for t2 in range(12, 16):
                proj_chunk(t2, evict_on_act=True)

    nc.finalize()
    return nc


def _get_nc():
    if "nc" not in _CACHE:
        _CACHE["nc"] = _build()
    return _CACHE["nc"]


def kernel(x, W_qkv, b_qkv, W_proj, b_proj):
    from concourse.bass_utils import run_bass_kernel_spmd

    x = np.asarray(x, dtype=np.float32)
    W_qkv = np.asarray(W_qkv, dtype=np.float32)
    b_qkv = np.asarray(b_qkv, dtype=np.float32)
    W_proj = np.asarray(W_proj, dtype=np.float32)
    b_proj = np.asarray(b_proj, dtype=np.float32)

    in_maps = []
    for core in range(N_CORES):
        b = core // 2
        h2 = core % 2
        o = 512 * h2
        xt = np.ascontiguousarray(x[b].T).astype(np.float16)
        wq = W_qkv[:, o : o + 512]
        wk = W_qkv[:, 1024 + o : 1024 + o + 512]
        wqk = np.ascontiguousarray(np.concatenate([wq, wk], axis=1)).astype(np.float16)
        wv = np.ascontiguousarray(W_qkv[:, 2048 + o : 2048 + o + 512]).astype(np.float16)
        wp = np.ascontiguousarray(W_proj[o : o + 512, :]).astype(np.float16)
        bq = b_qkv[o : o + 512]
        bk = b_qkv[1024 + o : 1024 + o + 512]
        bqk = np.ascontiguousarray(
            np.concatenate([bq, bk]).reshape(8, 128).T
        )
        bv = np.ascontiguousarray(b_qkv[2048 + o : 2048 + o + 512].reshape(1, 512)).astype(np.float16)
        in_maps.append(
            {"xt": xt, "wqk": wqk, "wv": wv, "wp": wp, "bqk": bqk, "bv": bv}
        )

    nc = _get_nc()
    kwargs = {}
    if os.environ.get("BASS_KERNEL_TRACE"):
        kwargs["trace"] = True
    res = run_bass_kernel_spmd(nc, in_maps, core_ids=list(range(N_CORES)), **kwargs)
    _CACHE["last_results"] = res

    out = np.empty((B, T, N_EMBD), dtype=np.float32)
    for b in range(B):
        out[b] = (
            res.results[2 * b]["out_p"]
            + res.results[2 * b + 1]["out_p"]
            + b_proj[None, :]
        )
    return out



# revision 4
# speedup vs baseline: 1.0261x; 1.0261x over previous
"""Causal self-attention Bass kernel for TRN2, 8 NeuronCores.

Sharding: data-parallel over batch (B=4) x tensor-parallel over head halves
(2 groups of 8 heads) = 8 shards, Megatron-style. Each core computes its
batch's qkv projection for its 8 heads, causal attention, and a partial
output projection (its heads' rows of W_proj). The host sums the two
partials per batch and adds b_proj.

All matmul operands are fp16 (full-rate 1 cycle/row on the PE, fp32 PSUM
accumulation). Output partials are written fp16 (halves HBM writes; the
host sums in fp32).

Schedule (v2): the PE stream is kept dense end-to-end.
  - All 16 V chunks + QK(0) run upfront, paced by the input DMA stream
    (xt0..xt3 loaded before wqk so the PE always has V work while the
    2MB wqk transfer is in flight).
  - Attention blocks use a lookahead-1 software pipeline: emit
    sc(si), pv(si-2), [filler] per slot so the exp (ACT) of a chunk is
    hidden behind the next chunk's score matmuls.
  - Rounds tj=2,3 spread output-projection matmuls inside blocks (one
    per slot) to cover the ~90ns/slot ACT-over-PE deficit; qk chunks
    for round tj+1 sit at block tails.
  - The causal mask multiply only touches the 128 diagonal columns.
  - Tail: proj chunks 12..15 follow the last block; their pair-0..2
    matmuls don't depend on the last normalize, covering its latency.
"""

import math
import os

import numpy as np

import concourse.bass as bass
import concourse.mybir as mybir
from concourse import bacc
from concourse.tile import TileContext

F32 = mybir.dt.float32
F16 = mybir.dt.float16

N_EMBD = 1024
N_HEAD = 16
D = 64
B = 4
T = 2048
N_CORES = 8
PAIRS = 4          # head pairs per core (8 heads)
TJ = T // 512      # 512-wide t super-chunks
SJ = T // 128      # 128-wide s chunks
SCALE = 1.0 / math.sqrt(D)

_CACHE = {}


def _build():
    nc = bacc.Bacc()

    xt_d = nc.declare_dram_parameter("xt", [N_EMBD, T], F16, isOutput=False)
    wqk_d = nc.declare_dram_parameter("wqk", [N_EMBD, 1024], F16, isOutput=False)
    wv_d = nc.declare_dram_parameter("wv", [N_EMBD, 512], F16, isOutput=False)
    wp_d = nc.declare_dram_parameter("wp", [512, N_EMBD], F16, isOutput=False)
    bqk_d = nc.declare_dram_parameter("bqk", [128, 8], F32, isOutput=False)
    bv_d = nc.declare_dram_parameter("bv", [1, 512], F16, isOutput=False)
    out_d = nc.declare_dram_parameter("out_p", [T, N_EMBD], F16, isOutput=True)

    with TileContext(nc) as tc:
        with (
            tc.tile_pool(name="const", bufs=1) as cpool,
            tc.tile_pool(name="w", bufs=1) as wpool,
            tc.tile_pool(name="xt", bufs=1) as xpool,
            tc.tile_pool(name="qkt", bufs=1) as qkpool,
            tc.tile_pool(name="v", bufs=1) as vpool,
            tc.tile_pool(name="e", bufs=8) as epool,
            tc.tile_pool(name="octp", bufs=1) as octpool,
            tc.tile_pool(name="misc", bufs=3) as mpool,
            tc.tile_pool(name="outp", bufs=3) as opool,
            tc.tile_pool(name="ps", bufs=2, space="PSUM") as pspool,
            tc.tile_pool(name="pv", bufs=4, space="PSUM") as pvpool,
        ):
            # ---- constants ----
            ones_f = cpool.tile([1, 128], F32, tag="ones_f")
            nc.vector.memset(ones_f, 1.0)
            ones_r = cpool.tile([1, 128], F16, tag="ones_r")
            nc.vector.tensor_copy(ones_r, ones_f)
            ones8 = cpool.tile([128, 8], F32, tag="ones8")
            nc.vector.memset(ones8, 1.0)
            bqk_t = cpool.tile([128, 8], F32, tag="bqk")
            nc.sync.dma_start(out=bqk_t, in_=bqk_d[:, :])
            bv_t = cpool.tile([1, 512], F16, tag="bv")
            nc.sync.dma_start(out=bv_t, in_=bv_d[:, :])

            # bvb = b_v broadcast to [128, 512] via K=1 matmul
            ps_bvb = pvpool.tile([128, 512], F32, tag="pv")
            nc.tensor.matmul(
                ps_bvb, lhsT=ones_r[0:1, :], rhs=bv_t, start=True, stop=True
            )
            bvb = cpool.tile([128, 512], F32, tag="bvb")
            nc.vector.tensor_copy(bvb, ps_bvb)

            # triangular mask for the diagonal 128x128 block: keep where f >= p
            mask = cpool.tile([128, 128], F16, tag="mask")
            nc.vector.memset(mask, 1.0)
            nc.gpsimd.affine_select(
                out=mask, in_=mask, compare_op=mybir.AluOpType.is_ge, fill=0.0,
                base=0, pattern=[[1, 128]], channel_multiplier=-1,
            )

            # ---- input DMA: wv+xt0 interleaved, xt1..3, wqk, wp ----
            wqk = []
            wv = []
            wp = []
            xts = [[None] * 8 for _ in range(TJ)]
            for c in range(8):
                t = wpool.tile([128, 512], F16, tag=f"wv{c}")
                nc.sync.dma_start(out=t, in_=wv_d[128 * c : 128 * c + 128, :])
                wv.append(t)
                t2_ = xpool.tile([128, 512], F16, tag=f"xt0_{c}")
                nc.sync.dma_start(out=t2_, in_=xt_d[128 * c : 128 * c + 128, 0:512])
                xts[0][c] = t2_
            for tj in range(1, TJ):
                for c in range(8):
                    t = xpool.tile([128, 512], F16, tag=f"xt{tj}_{c}")
                    nc.sync.dma_start(
                        out=t,
                        in_=xt_d[128 * c : 128 * c + 128, 512 * tj : 512 * tj + 512],
                    )
                    xts[tj][c] = t
            for c in range(8):
                t = wpool.tile([128, 1024], F16, tag=f"wqk{c}")
                nc.sync.dma_start(out=t, in_=wqk_d[128 * c : 128 * c + 128, :])
                wqk.append(t)
            for p in range(PAIRS):
                t = wpool.tile([128, 1024], F16, tag=f"wp{p}")
                nc.sync.dma_start(out=t, in_=wp_d[128 * p : 128 * p + 128, :])
                wp.append(t)

            QT = [[None] * TJ for _ in range(PAIRS)]
            KT = [[None] * TJ for _ in range(PAIRS)]
            V = [None] * SJ
            OCT = [[None] * TJ for _ in range(PAIRS)]

            def v_chunk(tj, sj):
                s_idx = 4 * tj + sj
                pv = pvpool.tile([128, 512], F32, tag="pv")
                for c in range(8):
                    nc.tensor.matmul(
                        pv,
                        lhsT=xts[tj][c][:, 128 * sj : 128 * sj + 128],
                        rhs=wv[c],
                        start=(c == 0),
                        stop=(c == 7),
                    )
                vt = vpool.tile([128, 8, 65], F16, tag=f"v{s_idx}")
                nc.vector.tensor_add(
                    vt[:, :, 0:64],
                    pv.rearrange("p (h d) -> p h d", h=8),
                    bvb.rearrange("p (h d) -> p h d", h=8),
                )
                nc.vector.tensor_copy(
                    vt[:, :, 64:65], ones8.rearrange("p (h o) -> p h o", h=8)
                )
                V[s_idx] = vt

            def qk_chunk(tj, n):
                ps = pspool.tile([128, 1024], F32, tag="ps")
                for c in range(8):
                    nc.tensor.matmul(
                        ps[:, 0:512],
                        lhsT=wqk[c][:, 128 * n : 128 * n + 128],
                        rhs=xts[tj][c],
                        start=(c == 0),
                        stop=(c == 7),
                    )
                dst = qkpool.tile([128, 512], F16, tag=f"qk{n}_{tj}")
                nc.vector.tensor_scalar_add(dst, ps[:, 0:512], bqk_t[:, n : n + 1])
                if n < 4:
                    QT[n][tj] = dst
                else:
                    KT[n - 4][tj] = dst

            def proj_chunk_units(t2, evict_on_act=False):
                """Return a list of closures, each emitting one PE matmul (or
                the eviction+DMA) of the output projection for t-chunk t2."""
                tcj, k = t2 // 4, t2 % 4
                units = []
                state = {}

                def mk_mm(cj, pair):
                    def _u():
                        if pair == 0:
                            state[cj] = pvpool.tile(
                                [128, 512], F32, tag="pv", name=f"po{t2}_{cj}"
                            )
                        nc.tensor.matmul(
                            state[cj],
                            lhsT=OCT[pair][tcj][:, 128 * k : 128 * k + 128],
                            rhs=wp[pair][:, 512 * cj : 512 * cj + 512],
                            start=(pair == 0),
                            stop=(pair == 3),
                        )
                    return _u

                def mk_evict(cj):
                    def _u():
                        ot = opool.tile(
                            [128, 512], F16, tag="out", name=f"ot{t2}_{cj}"
                        )
                        if evict_on_act:
                            nc.scalar.copy(ot, state[cj])
                        else:
                            nc.vector.tensor_copy(ot, state[cj])
                        nc.sync.dma_start(
                            out=out_d[
                                128 * t2 : 128 * t2 + 128, 512 * cj : 512 * cj + 512
                            ],
                            in_=ot,
                        )
                    return _u

                for cj in range(2):
                    for pair in range(PAIRS):
                        units.append(mk_mm(cj, pair))
                    units.append(mk_evict(cj))
                return units

            def attention_block(pair, tcj, fillers):
                """fillers: list of closures, each emitting ~1 PE matmul of
                independent work; popped one per si slot (from slot 1)."""
                nk = 4 * tcj + 4
                pv1 = pvpool.tile([128, 512], F32, tag="pv")
                pv2 = pvpool.tile([128, 512], F32, tag="pv")
                qt = QT[pair][tcj]
                ets = [None] * nk
                f0s = [None] * nk

                def emit_sc(si):
                    kt = KT[pair][si // 4]
                    koff = 128 * (si % 4)
                    f0 = max(0, 128 * (si - 4 * tcj))
                    ps = pspool.tile([128, 1024], F32, tag="ps")
                    psv = ps.rearrange("p (g f) -> p g f", g=2)
                    nc.tensor.matmul(
                        psv[:, 0, f0:512],
                        lhsT=kt[0:64, koff : koff + 128],
                        rhs=qt[0:64, f0:512],
                        start=True,
                        stop=True,
                        tile_position=(0, 0),
                    )
                    nc.tensor.matmul(
                        psv[:, 1, f0:512],
                        lhsT=kt[64:128, koff : koff + 128],
                        rhs=qt[64:128, f0:512],
                        start=True,
                        stop=True,
                        tile_position=(64, 0),
                    )
                    et = epool.tile([128, 2, 512], F16, tag="e")
                    nc.scalar.activation(
                        out=et[:, :, f0:512],
                        in_=psv[:, :, f0:512],
                        func=mybir.ActivationFunctionType.Exp,
                        scale=SCALE,
                    )
                    if si >= 4 * tcj:
                        # diagonal chunk: zero the upper triangle of the
                        # 128-wide diagonal column band (cols > f0+127 are
                        # fully kept, cols < f0 fully masked / not computed)
                        nc.vector.tensor_mul(
                            et[:, 0, f0 : f0 + 128], et[:, 0, f0 : f0 + 128], mask
                        )
                        nc.vector.tensor_mul(
                            et[:, 1, f0 : f0 + 128], et[:, 1, f0 : f0 + 128], mask
                        )
                    ets[si], f0s[si] = et, f0

                def emit_pv(si):
                    et, f0 = ets[si], f0s[si]
                    h1 = 2 * pair
                    h2 = 2 * pair + 1
                    nc.tensor.matmul(
                        pv1[0:65, f0:512],
                        lhsT=V[si][:, h1, :],
                        rhs=et[:, 0, f0:512],
                        start=(si == 0),
                        stop=(si == nk - 1),
                    )
                    nc.tensor.matmul(
                        pv2[0:65, f0:512],
                        lhsT=V[si][:, h2, :],
                        rhs=et[:, 1, f0:512],
                        start=(si == 0),
                        stop=(si == nk - 1),
                    )

                for si in range(nk):
                    emit_sc(si)
                    if si >= 2:
                        emit_pv(si - 2)
                    if si >= 1 and fillers:
                        fillers.pop(0)()
                emit_pv(nk - 2)
                emit_pv(nk - 1)

                # normalize: O[d, t] / Z[t]  (Z in psum row 64)
                oct_t = octpool.tile([128, 512], F16, tag=f"oct{pair}_{tcj}")
                OCT[pair][tcj] = oct_t
                for g, pv in enumerate((pv1, pv2)):
                    rz = mpool.tile([1, 512], F32, tag="rz")
                    nc.vector.tensor_copy(rz, pv[64:65, :])
                    nc.vector.reciprocal_approx_fast(out=rz, in_=rz)
                    rzb = mpool.tile([64, 512], F32, tag="rzb")
                    nc.gpsimd.partition_broadcast(rzb, rz)
                    nc.vector.tensor_mul(
                        oct_t[64 * g : 64 * g + 64, :], pv[0:64, :], rzb
                    )

            # ---- emission ----
            for tj in range(TJ):
                for sj in range(4):
                    v_chunk(tj, sj)
            for n in (0, 4, 1, 5, 2, 6, 3, 7):
                qk_chunk(0, n)

            # intra-block fillers: proj chunks for rounds tj>=2
            for tj in range(TJ):
                if tj == 2:
                    proj_units = (
                        proj_chunk_units(0) + proj_chunk_units(1)
                        + proj_chunk_units(2) + proj_chunk_units(3)
                    )
                elif tj == 3:
                    proj_units = []
                    for t2 in range(4, 12):
                        proj_units += proj_chunk_units(t2)
                else:
                    proj_units = []
                per_block = (len(proj_units) + PAIRS - 1) // PAIRS
                for pair in range(PAIRS):
                    fillers = proj_units[:per_block]
                    del proj_units[:per_block]
                    attention_block(pair, tj, fillers)
                    # run any fillers the block didn't consume
                    for f in fillers:
                        f()
                    if tj + 1 < TJ:
                        qk_chunk(tj + 1, pair)
                        qk_chunk(tj + 1, 4 + pair)

            # tail: t2=12..15 need the last round's OCT; the pair-0..2
            # matmuls of chunk 12 cover the final normalize latency.
            for t2 in range(12, 16):
                for u in proj_chunk_units(t2, evict_on_act=True):
                    u()

    nc.finalize()
    return nc


def _get_nc():
    if "nc" not in _CACHE:
        _CACHE["nc"] = _build()
    return _CACHE["nc"]


def kernel(x, W_qkv, b_qkv, W_proj, b_proj):
    from concourse.bass_utils import run_bass_kernel_spmd

    x = np.asarray(x, dtype=np.float32)
    W_qkv = np.asarray(W_qkv, dtype=np.float32)
    b_qkv = np.asarray(b_qkv, dtype=np.float32)
    W_proj = np.asarray(W_proj, dtype=np.float32)
    b_proj = np.asarray(b_proj, dtype=np.float32)

    in_maps = []
    for core in range(N_CORES):
        b = core // 2
        h2 = core % 2
        o = 512 * h2
        xt = np.ascontiguousarray(x[b].T).astype(np.float16)
        wq = W_qkv[:, o : o + 512]
        wk = W_qkv[:, 1024 + o : 1024 + o + 512]
        wqk = np.ascontiguousarray(np.concatenate([wq, wk], axis=1)).astype(np.float16)
        wv = np.ascontiguousarray(W_qkv[:, 2048 + o : 2048 + o + 512]).astype(np.float16)
        wp = np.ascontiguousarray(W_proj[o : o + 512, :]).astype(np.float16)
        bq = b_qkv[o : o + 512]
        bk = b_qkv[1024 + o : 1024 + o + 512]
        bqk = np.ascontiguousarray(
            np.concatenate([bq, bk]).reshape(8, 128).T
        )
        bv = np.ascontiguousarray(b_qkv[2048 + o : 2048 + o + 512].reshape(1, 512)).astype(np.float16)
        in_maps.append(
            {"xt": xt, "wqk": wqk, "wv": wv, "wp": wp, "bqk": bqk, "bv": bv}
        )

    nc = _get_nc()
    kwargs = {}
    if os.environ.get("BASS_KERNEL_TRACE"):
        kwargs["trace"] = True
    res = run_bass_kernel_spmd(nc, in_maps, core_ids=list(range(N_CORES)), **kwargs)
    _CACHE["last_results"] = res

    out = np.empty((B, T, N_EMBD), dtype=np.float32)
    for b in range(B):
        out[b] = (
            res.results[2 * b]["out_p"].astype(np.float32)
            + res.results[2 * b + 1]["out_p"].astype(np.float32)
            + b_proj[None, :]
        )
    return out


# revision 7
# speedup vs baseline: 1.0705x; 1.0432x over previous
"""Causal self-attention Bass kernel for TRN2, 8 NeuronCores.

Sharding: data-parallel over batch (B=4) x tensor-parallel over head halves
(2 groups of 8 heads) = 8 shards, Megatron-style. Each core computes its
batch's qkv projection for its 8 heads, causal attention, and a partial
output projection (its heads' rows of W_proj). The host sums the two
partials per batch and adds b_proj.

All matmul operands are fp16 (full-rate 1 cycle/row on the PE, fp32 PSUM
accumulation). Output partials are written fp16 (halves HBM writes; the
host sums in fp32).

Schedule (v2): the PE stream is kept dense end-to-end.
  - All 16 V chunks + QK(0) run upfront, paced by the input DMA stream
    (xt0..xt3 loaded before wqk so the PE always has V work while the
    2MB wqk transfer is in flight).
  - Attention blocks use a lookahead-1 software pipeline: emit
    sc(si), pv(si-2), [filler] per slot so the exp (ACT) of a chunk is
    hidden behind the next chunk's score matmuls.
  - Rounds tj=2,3 spread output-projection matmuls inside blocks (one
    per slot) to cover the ~90ns/slot ACT-over-PE deficit; qk chunks
    for round tj+1 sit at block tails.
  - The causal mask multiply only touches the 128 diagonal columns.
  - Tail: proj chunks 12..15 follow the last block; their pair-0..2
    matmuls don't depend on the last normalize, covering its latency.
"""

import math
import os

import numpy as np

import concourse.bass as bass
import concourse.mybir as mybir
from concourse import bacc
from concourse.tile import TileContext

F32 = mybir.dt.float32
F16 = mybir.dt.float16

N_EMBD = 1024
N_HEAD = 16
D = 64
B = 4
T = 2048
N_CORES = 8
PAIRS = 4          # head pairs per core (8 heads)
TJ = T // 512      # 512-wide t super-chunks
SJ = T // 128      # 128-wide s chunks
SCALE = 1.0 / math.sqrt(D)

_CACHE = {}


def _build():
    nc = bacc.Bacc()

    xt_d = nc.declare_dram_parameter("xt", [N_EMBD, T], F16, isOutput=False)
    wqk_d = nc.declare_dram_parameter("wqk", [N_EMBD, 1024], F16, isOutput=False)
    wv_d = nc.declare_dram_parameter("wv", [N_EMBD, 512], F16, isOutput=False)
    wp_d = nc.declare_dram_parameter("wp", [512, N_EMBD], F16, isOutput=False)
    bqk_d = nc.declare_dram_parameter("bqk", [128, 8], F32, isOutput=False)
    bv_d = nc.declare_dram_parameter("bv", [1, 512], F16, isOutput=False)
    out_d = nc.declare_dram_parameter("out_p", [T, N_EMBD], F16, isOutput=True)

    with TileContext(nc) as tc:
        with (
            tc.tile_pool(name="const", bufs=1) as cpool,
            tc.tile_pool(name="w", bufs=1) as wpool,
            tc.tile_pool(name="xt", bufs=1) as xpool,
            tc.tile_pool(name="qkt", bufs=1) as qkpool,
            tc.tile_pool(name="v", bufs=1) as vpool,
            tc.tile_pool(name="e", bufs=8) as epool,
            tc.tile_pool(name="octp", bufs=1) as octpool,
            tc.tile_pool(name="misc", bufs=3) as mpool,
            tc.tile_pool(name="outp", bufs=3) as opool,
            tc.tile_pool(name="ps", bufs=2, space="PSUM") as pspool,
            tc.tile_pool(name="pv", bufs=4, space="PSUM") as pvpool,
        ):
            # ---- constants ----
            ones_f = cpool.tile([1, 128], F32, tag="ones_f")
            nc.vector.memset(ones_f, 1.0)
            ones_r = cpool.tile([1, 128], F16, tag="ones_r")
            nc.vector.tensor_copy(ones_r, ones_f)
            ones8 = cpool.tile([128, 8], F32, tag="ones8")
            nc.vector.memset(ones8, 1.0)
            bqk_t = cpool.tile([128, 8], F32, tag="bqk")
            nc.sync.dma_start(out=bqk_t, in_=bqk_d[:, :])
            bv_t = cpool.tile([1, 512], F16, tag="bv")
            nc.sync.dma_start(out=bv_t, in_=bv_d[:, :])

            # bvb = b_v broadcast to [128, 512] via K=1 matmul
            ps_bvb = pvpool.tile([128, 512], F32, tag="pv")
            nc.tensor.matmul(
                ps_bvb, lhsT=ones_r[0:1, :], rhs=bv_t, start=True, stop=True
            )
            bvb = cpool.tile([128, 512], F32, tag="bvb")
            nc.vector.tensor_copy(bvb, ps_bvb)

            # triangular mask for the diagonal 128x128 block: keep where f >= p
            mask = cpool.tile([128, 128], F16, tag="mask")
            nc.vector.memset(mask, 1.0)
            nc.gpsimd.affine_select(
                out=mask, in_=mask, compare_op=mybir.AluOpType.is_ge, fill=0.0,
                base=0, pattern=[[1, 128]], channel_multiplier=-1,
            )

            # ---- input DMA: wv+xt0 interleaved, xt1..3, wqk, wp ----
            wqk = []
            wv = []
            wp = []
            xts = [[None] * 8 for _ in range(TJ)]
            for c in range(8):
                t = wpool.tile([128, 512], F16, tag=f"wv{c}")
                nc.sync.dma_start(out=t, in_=wv_d[128 * c : 128 * c + 128, :])
                wv.append(t)
                t2_ = xpool.tile([128, 512], F16, tag=f"xt0_{c}")
                nc.sync.dma_start(out=t2_, in_=xt_d[128 * c : 128 * c + 128, 0:512])
                xts[0][c] = t2_
            for tj in range(1, TJ):
                for c in range(8):
                    t = xpool.tile([128, 512], F16, tag=f"xt{tj}_{c}")
                    nc.sync.dma_start(
                        out=t,
                        in_=xt_d[128 * c : 128 * c + 128, 512 * tj : 512 * tj + 512],
                    )
                    xts[tj][c] = t
            for c in range(8):
                t = wpool.tile([128, 1024], F16, tag=f"wqk{c}")
                nc.sync.dma_start(out=t, in_=wqk_d[128 * c : 128 * c + 128, :])
                wqk.append(t)
            for p in range(PAIRS):
                t = wpool.tile([128, 1024], F16, tag=f"wp{p}")
                nc.sync.dma_start(out=t, in_=wp_d[128 * p : 128 * p + 128, :])
                wp.append(t)

            QT = [[None] * TJ for _ in range(PAIRS)]
            KT = [[None] * TJ for _ in range(PAIRS)]
            V = [None] * SJ
            OCT = [[None] * TJ for _ in range(PAIRS)]

            def v_chunk(tj, sj):
                s_idx = 4 * tj + sj
                pv = pvpool.tile([128, 512], F32, tag="pv")
                for c in range(8):
                    nc.tensor.matmul(
                        pv,
                        lhsT=xts[tj][c][:, 128 * sj : 128 * sj + 128],
                        rhs=wv[c],
                        start=(c == 0),
                        stop=(c == 7),
                    )
                vt = vpool.tile([128, 8, 65], F16, tag=f"v{s_idx}")
                nc.vector.tensor_add(
                    vt[:, :, 0:64],
                    pv.rearrange("p (h d) -> p h d", h=8),
                    bvb.rearrange("p (h d) -> p h d", h=8),
                )
                nc.vector.tensor_copy(
                    vt[:, :, 64:65], ones8.rearrange("p (h o) -> p h o", h=8)
                )
                V[s_idx] = vt

            def qk_chunk(tj, n):
                ps = pspool.tile([128, 1024], F32, tag="ps")
                for c in range(8):
                    nc.tensor.matmul(
                        ps[:, 0:512],
                        lhsT=wqk[c][:, 128 * n : 128 * n + 128],
                        rhs=xts[tj][c],
                        start=(c == 0),
                        stop=(c == 7),
                    )
                dst = qkpool.tile([128, 512], F16, tag=f"qk{n}_{tj}")
                nc.vector.tensor_scalar_add(dst, ps[:, 0:512], bqk_t[:, n : n + 1])
                if n < 4:
                    QT[n][tj] = dst
                else:
                    KT[n - 4][tj] = dst

            def proj_chunk_units(t2, evict_on_act=False):
                """Return a list of closures, each emitting one PE matmul (or
                the eviction+DMA) of the output projection for t-chunk t2."""
                tcj, k = t2 // 4, t2 % 4
                units = []
                state = {}

                def mk_mm(cj, pair):
                    def _u():
                        if pair == 0:
                            state[cj] = pvpool.tile(
                                [128, 512], F32, tag="pv", name=f"po{t2}_{cj}"
                            )
                        nc.tensor.matmul(
                            state[cj],
                            lhsT=OCT[pair][tcj][:, 128 * k : 128 * k + 128],
                            rhs=wp[pair][:, 512 * cj : 512 * cj + 512],
                            start=(pair == 0),
                            stop=(pair == 3),
                        )
                    return _u

                def mk_evict(cj):
                    def _u():
                        ot = opool.tile(
                            [128, 512], F16, tag="out", name=f"ot{t2}_{cj}"
                        )
                        if evict_on_act:
                            nc.scalar.copy(ot, state[cj])
                        else:
                            nc.vector.tensor_copy(ot, state[cj])
                        nc.sync.dma_start(
                            out=out_d[
                                128 * t2 : 128 * t2 + 128, 512 * cj : 512 * cj + 512
                            ],
                            in_=ot,
                        )
                    return _u

                for cj in range(2):
                    for pair in range(PAIRS):
                        units.append(mk_mm(cj, pair))
                    units.append(mk_evict(cj))
                return units

            def attention_block(pair, tcj, fillers):
                """fillers: list of closures, each emitting ~1 PE matmul of
                independent work; popped one per si slot (from slot 1)."""
                nk = 4 * tcj + 4
                pv1 = pvpool.tile([128, 512], F32, tag="pv")
                pv2 = pvpool.tile([128, 512], F32, tag="pv")
                qt = QT[pair][tcj]
                ets = [None] * nk
                f0s = [None] * nk

                def emit_sc(si):
                    kt = KT[pair][si // 4]
                    koff = 128 * (si % 4)
                    f0 = max(0, 128 * (si - 4 * tcj))
                    ps = pspool.tile([128, 1024], F32, tag="ps")
                    psv = ps.rearrange("p (g f) -> p g f", g=2)
                    nc.tensor.matmul(
                        psv[:, 0, f0:512],
                        lhsT=kt[0:64, koff : koff + 128],
                        rhs=qt[0:64, f0:512],
                        start=True,
                        stop=True,
                        tile_position=(0, 0),
                    )
                    nc.tensor.matmul(
                        psv[:, 1, f0:512],
                        lhsT=kt[64:128, koff : koff + 128],
                        rhs=qt[64:128, f0:512],
                        start=True,
                        stop=True,
                        tile_position=(64, 0),
                    )
                    et = epool.tile([128, 2, 512], F16, tag="e")
                    nc.scalar.activation(
                        out=et[:, :, f0:512],
                        in_=psv[:, :, f0:512],
                        func=mybir.ActivationFunctionType.Exp,
                        scale=SCALE,
                    )
                    if si >= 4 * tcj:
                        # diagonal chunk: zero the upper triangle of the
                        # 128-wide diagonal column band (cols > f0+127 are
                        # fully kept, cols < f0 fully masked / not computed)
                        nc.vector.tensor_mul(
                            et[:, 0, f0 : f0 + 128], et[:, 0, f0 : f0 + 128], mask
                        )
                        nc.vector.tensor_mul(
                            et[:, 1, f0 : f0 + 128], et[:, 1, f0 : f0 + 128], mask
                        )
                    ets[si], f0s[si] = et, f0

                def emit_pv(si):
                    et, f0 = ets[si], f0s[si]
                    h1 = 2 * pair
                    h2 = 2 * pair + 1
                    nc.tensor.matmul(
                        pv1[0:65, f0:512],
                        lhsT=V[si][:, h1, :],
                        rhs=et[:, 0, f0:512],
                        start=(si == 0),
                        stop=(si == nk - 1),
                    )
                    nc.tensor.matmul(
                        pv2[0:65, f0:512],
                        lhsT=V[si][:, h2, :],
                        rhs=et[:, 1, f0:512],
                        start=(si == 0),
                        stop=(si == nk - 1),
                    )

                for si in range(nk):
                    emit_sc(si)
                    if si >= 2:
                        emit_pv(si - 2)
                    if si >= 1 and fillers:
                        fillers.pop(0)()
                emit_pv(nk - 2)
                emit_pv(nk - 1)

                def normalize():
                    # O[d, t] / Z[t]  (Z in psum row 64); deferred so the
                    # qk-chunk evictions (which free the score PSUM bufs the
                    # next block needs) run first on the Vector engine.
                    oct_t = octpool.tile(
                        [128, 512], F16, tag=f"oct{pair}_{tcj}",
                        name=f"oct{pair}_{tcj}",
                    )
                    OCT[pair][tcj] = oct_t
                    rzs = []
                    for g, pv in enumerate((pv1, pv2)):
                        rz = mpool.tile([1, 512], F32, tag="rz", name=f"rz{g}")
                        nc.vector.tensor_copy(rz, pv[64:65, :])
                        nc.vector.reciprocal_approx_fast(out=rz, in_=rz)
                        rzs.append(rz)
                    rzbs = []
                    for g in range(2):
                        rzb = mpool.tile([64, 512], F32, tag="rzb", name=f"rzb{g}")
                        nc.gpsimd.partition_broadcast(rzb, rzs[g])
                        rzbs.append(rzb)
                    for g, pv in enumerate((pv1, pv2)):
                        nc.vector.tensor_mul(
                            oct_t[64 * g : 64 * g + 64, :], pv[0:64, :], rzbs[g]
                        )

                return normalize

            # ---- emission ----
            for tj in range(TJ):
                for sj in range(4):
                    v_chunk(tj, sj)
            for n in (0, 4, 1, 5, 2, 6, 3, 7):
                qk_chunk(0, n)

            # intra-block fillers: proj chunks for rounds tj>=2
            for tj in range(TJ):
                if tj == 2:
                    proj_units = (
                        proj_chunk_units(0) + proj_chunk_units(1)
                        + proj_chunk_units(2) + proj_chunk_units(3)
                    )
                elif tj == 3:
                    proj_units = []
                    for t2 in range(4, 12):
                        proj_units += proj_chunk_units(t2)
                else:
                    proj_units = []
                per_block = (len(proj_units) + PAIRS - 1) // PAIRS
                for pair in range(PAIRS):
                    fillers = proj_units[:per_block]
                    del proj_units[:per_block]
                    normalize = attention_block(pair, tj, fillers)
                    # run any fillers the block didn't consume
                    for f in fillers:
                        f()
                    if tj + 1 < TJ:
                        qk_chunk(tj + 1, pair)
                        qk_chunk(tj + 1, 4 + pair)
                    normalize()

            # tail: t2=12..15 need the last round's OCT. Stagger the pair-3
            # matmuls (which wait on the final normalize) behind the pair-0..2
            # matmuls of two chunks at a time.
            for cj in range(2):
                for t2a, t2b in ((12, 13), (14, 15)):
                    pos = {}
                    for t2 in (t2a, t2b):
                        tcj, k = t2 // 4, t2 % 4
                        po = pvpool.tile(
                            [128, 512], F32, tag="pv", name=f"pot{t2}_{cj}"
                        )
                        pos[t2] = po
                        for pair in range(3):
                            nc.tensor.matmul(
                                po,
                                lhsT=OCT[pair][tcj][:, 128 * k : 128 * k + 128],
                                rhs=wp[pair][:, 512 * cj : 512 * cj + 512],
                                start=(pair == 0),
                                stop=False,
                            )
                    for t2 in (t2a, t2b):
                        tcj, k = t2 // 4, t2 % 4
                        nc.tensor.matmul(
                            pos[t2],
                            lhsT=OCT[3][tcj][:, 128 * k : 128 * k + 128],
                            rhs=wp[3][:, 512 * cj : 512 * cj + 512],
                            start=False,
                            stop=True,
                        )
                        ot = opool.tile(
                            [128, 512], F16, tag="out", name=f"otl{t2}_{cj}"
                        )
                        nc.scalar.copy(ot, pos[t2])
                        nc.sync.dma_start(
                            out=out_d[
                                128 * t2 : 128 * t2 + 128, 512 * cj : 512 * cj + 512
                            ],
                            in_=ot,
                        )

    nc.finalize()
    return nc


def _get_nc():
    if "nc" not in _CACHE:
        _CACHE["nc"] = _build()
    return _CACHE["nc"]


def kernel(x, W_qkv, b_qkv, W_proj, b_proj):
    from concourse.bass_utils import run_bass_kernel_spmd

    x = np.asarray(x, dtype=np.float32)
    W_qkv = np.asarray(W_qkv, dtype=np.float32)
    b_qkv = np.asarray(b_qkv, dtype=np.float32)
    W_proj = np.asarray(W_proj, dtype=np.float32)
    b_proj = np.asarray(b_proj, dtype=np.float32)

    in_maps = []
    for core in range(N_CORES):
        b = core // 2
        h2 = core % 2
        o = 512 * h2
        xt = np.ascontiguousarray(x[b].T).astype(np.float16)
        wq = W_qkv[:, o : o + 512]
        wk = W_qkv[:, 1024 + o : 1024 + o + 512]
        wqk = np.ascontiguousarray(np.concatenate([wq, wk], axis=1)).astype(np.float16)
        wv = np.ascontiguousarray(W_qkv[:, 2048 + o : 2048 + o + 512]).astype(np.float16)
        wp = np.ascontiguousarray(W_proj[o : o + 512, :]).astype(np.float16)
        bq = b_qkv[o : o + 512]
        bk = b_qkv[1024 + o : 1024 + o + 512]
        bqk = np.ascontiguousarray(
            np.concatenate([bq, bk]).reshape(8, 128).T
        )
        bv = np.ascontiguousarray(b_qkv[2048 + o : 2048 + o + 512].reshape(1, 512)).astype(np.float16)
        in_maps.append(
            {"xt": xt, "wqk": wqk, "wv": wv, "wp": wp, "bqk": bqk, "bv": bv}
        )

    nc = _get_nc()
    kwargs = {}
    if os.environ.get("BASS_KERNEL_TRACE"):
        kwargs["trace"] = True
    res = run_bass_kernel_spmd(nc, in_maps, core_ids=list(range(N_CORES)), **kwargs)
    _CACHE["last_results"] = res

    out = np.empty((B, T, N_EMBD), dtype=np.float32)
    for b in range(B):
        out[b] = (
            res.results[2 * b]["out_p"].astype(np.float32)
            + res.results[2 * b + 1]["out_p"].astype(np.float32)
            + b_proj[None, :]
        )
    return out
